# revision 9
# baseline (speedup 1.0000x reference)
"""Trainium2 Bass kernel for nn_Block_69578470195514 (dense transformer block).

kernel(**inputs): FULL inputs (B=8,T=1024,D=1024,H=16) -> FULL [8,1024,1024] f32.
Sharding: data-parallel over batch, core b handles batch element b. No collectives.

Per-core pipeline (fp8 = TRN e4m3 via DoubleRow double-pumped matmuls):
  A: x load -> LN1 (stats DVE, sqrt ACT, apply ACT) -> PE transpose -> xnT fp8 [d,t]
  B: per head-pair pr: qT/kT = Wq8^T xnT (fp8 DoubleRow, dequant on copy),
     v = xnT^T Wv8; scoresT[s,t] bf16 (K=64, 2 heads in PE row quadrants),
     exp on ACT (full-row, accum_out row sums), vp = v*r, attT[e,t] += vp^T wexp
     (bf16, 2 heads in PE col quadrants). attT stored fp8 (x16).
     Next pair's qkv matmuls interleave as PE filler during exp waits.
  C: proj fp8 DoubleRow: x2 = x + attT^T Wp8 -> DRAM scratch; LN2 -> xn2T bf16
  D/E interleaved by t-halves (h half fits SBUF):
     FF1(half) -> h0 bf16 [f, 512] -> FF2(4 t-tiles): out = x2 + h0^T W2
Weights pow2-scaled on host for fp8; dequant folded into psum->sbuf copies.
"""

import numpy as np
import ml_dtypes

BF16 = ml_dtypes.bfloat16
F8E4 = ml_dtypes.float8_e4m3  # IEEE e4m3: max 240 == TRN FP8_EXP4

P = 128
B, T, D, H = 8, 1024, 1024, 16
DH = D // H
F = 4 * D
NT = T // P      # 8 token tiles
ND = D // P      # 8 feature tiles
NF = F // P      # 32 ff tiles
NPR = H // 2     # 8 head pairs
NF8 = 8          # leading FF1 f-tiles computed in fp8 DoubleRow
EPS = 1e-3
SCALE = float(D) ** -0.5
MASKVAL = -1.0e6
SATT = 16.0      # attT fp8 pre-scale

_cache = {}


def _split_multiwait_insts(nc, mybir):
    """This walrus build allows only 1 sync-wait per instruction. Hoist all
    but the last wait into single-wait InstEventSemaphore carriers."""
    for bb in nc.main_func.blocks:
        insts = list(bb.instructions)
        out = []
        changed = False
        for inst in insts:
            si = inst.sync_info
            if si is not None and si.on_wait and len(si.on_wait) > 1:
                waits = list(si.on_wait)
                for k, w in enumerate(waits[:-1]):
                    d = mybir.InstEventSemaphore(
                        name=f"{inst.name}_wsplit{k}", ins=[], outs=[]
                    )
                    d.engine = inst.engine
                    d.sync_info = mybir.SyncInfo(on_wait=[w], on_update=[])
                    out.append(d)
                inst.sync_info = mybir.SyncInfo(
                    on_wait=[waits[-1]], on_update=list(si.on_update)
                )
                changed = True
            out.append(inst)
        if changed:
            try:
                bb.instructions[:] = out
            except Exception:
                bb.instructions.clear()
                for i in out:
                    bb.add_instruction(i)


def _av_chunks(r0):
    """Column chunks for AV matmuls of s-tile starting at r0, split on PSUM
    bank boundaries (512 fp32)."""
    chunks = []
    for b0 in range(0, T, 512):
        lo = max(r0, b0)
        hi = b0 + 512
        if lo < hi:
            chunks.append((lo, hi))
    return chunks


def _pow2_scale(w):
    return float(2.0 ** np.floor(np.log2(240.0 / np.abs(w).max())))


def _to_f8(w, s):
    return np.clip(w * s, -240.0, 240.0).astype(F8E4)


def _prep_inputs(
    x, gamma1, beta1, Wq, Wk, Wv, Wp, bp, gamma2, beta2, W1, b1, W2, b2
):
    g1 = np.asarray(gamma1, np.float32)
    be1 = np.asarray(beta1, np.float32)
    g2 = np.asarray(gamma2, np.float32)
    be2 = np.asarray(beta2, np.float32)
    Wq2 = np.asarray(Wq, np.float32).transpose(1, 0, 2).reshape(D, D)
    Wk2 = np.asarray(Wk, np.float32).transpose(1, 0, 2).reshape(D, D)
    Wv2 = np.asarray(Wv, np.float32).transpose(1, 0, 2).reshape(D, D)
    Wpf = np.asarray(Wp, np.float32)
    W1f = np.asarray(W1, np.float32)
    W2f = np.asarray(W2, np.float32)

    # fold gamma into the LN-consumer weights (kernel computes (x-mu)*rsig)
    Wqg = Wq2 * g1[:, None]
    Wkg = Wk2 * g1[:, None]
    Wvg = Wv2 * g1[:, None]
    W1g = W1f * g2[:, None]

    sq = _pow2_scale(Wqg)
    sk = _pow2_scale(Wkg)
    sv = _pow2_scale(Wvg)
    sp = _pow2_scale(Wpf)

    # [p, pr, kd, e] = W[kd*128+p, pr*128+e]
    def qkv_layout(w, s):
        a = _to_f8(w, s).reshape(ND, P, NPR, P)  # [kd, p, pr, e]
        return np.ascontiguousarray(a.transpose(1, 2, 0, 3))

    # [p, kd, e] = W[kd*128+p, e]
    def kmaj_layout(w, dt_, ncol):
        a = np.asarray(w).reshape(-1, P, ncol)  # [kd, p, e]
        return np.ascontiguousarray(a.transpose(1, 0, 2).astype(dt_))

    # [p, mf, kd, f] = W1[kd*128+p, mf*128+f]
    w1a = W1g.astype(BF16).reshape(ND, P, NF, P)
    w1l = np.ascontiguousarray(w1a.transpose(1, 2, 0, 3))
    s1 = _pow2_scale(W1g[:, :NF8 * P])
    w18a = _to_f8(W1g[:, :NF8 * P], s1).reshape(ND, P, NF8, P)
    w18l = np.ascontiguousarray(w18a.transpose(1, 2, 0, 3))

    qb = be1 @ Wq2  # beta1 rides through the unfolded weights
    kb = be1 @ Wk2
    vb = be1 @ Wv2
    b1p = np.asarray(b1, np.float32) + be2 @ W1f
    bpf = np.asarray(bp, np.float32)
    b2f = np.asarray(b2, np.float32)

    common = {
        "wq8": qkv_layout(Wqg, sq),
        "wk8": qkv_layout(Wkg, sk),
        "wv8": qkv_layout(Wvg, sv),
        "wp8": kmaj_layout(np.clip(Wpf * sp, -240, 240), F8E4, D),
        "w1": w1l,
        "w18": w18l,
        "w2": kmaj_layout(W2f, BF16, D),
        "mask": np.where(
            np.arange(P)[None, :] < np.arange(P)[:, None], MASKVAL, 0.0
        ).astype(np.float32),
        "ident": np.eye(P, dtype=BF16),
    }
    flags = {
        "has_qkb": bool(np.any(qb) or np.any(kb)),
        "has_vb": bool(np.any(vb)),
        "has_b1": bool(np.any(b1p)),
        "has_bp": bool(np.any(bpf)),
        "has_b2": bool(np.any(b2f)),
    }
    if flags["has_qkb"]:
        common["qbt"] = np.ascontiguousarray(qb.reshape(NPR, P).T)
        common["kbt"] = np.ascontiguousarray(kb.reshape(NPR, P).T)
    if flags["has_vb"]:
        common["vbr"] = np.ascontiguousarray(vb.reshape(1, D))
    if flags["has_b1"]:
        common["b1t"] = np.ascontiguousarray(b1p.reshape(NF, P).T)
    if flags["has_bp"]:
        common["bpr"] = np.ascontiguousarray(bpf.reshape(1, D))
    if flags["has_b2"]:
        common["b2r"] = np.ascontiguousarray(b2f.reshape(1, D))
    scales = {"inv_q": 1.0 / sq, "inv_k": 1.0 / sk, "inv_v": 1.0 / sv,
              "inv_proj": 1.0 / (SATT * sp), "inv_1": 1.0 / s1}
    xs = np.asarray(x, np.float32)
    return xs, common, flags, scales


def _build(reps=1, flags=None, scales=None, debug=False, unroll=1):
    from contextlib import ExitStack

    import concourse.bass as bass
    import concourse.tile as tile
    import concourse.mybir as mybir

    flags = flags or {}
    f32 = mybir.dt.float32
    bf16 = mybir.dt.bfloat16
    f8 = mybir.dt.float8e4
    AF = mybir.ActivationFunctionType
    ALU = mybir.AluOpType
    DR = mybir.MatmulPerfMode.DoubleRow

    nc = bass.Bass()

    x_d = nc.dram_tensor("x", [T, D], f32, kind="ExternalInput")
    wq_d = nc.dram_tensor("wq8", [P, NPR, ND, P], f8, kind="ExternalInput")
    wk_d = nc.dram_tensor("wk8", [P, NPR, ND, P], f8, kind="ExternalInput")
    wv_d = nc.dram_tensor("wv8", [P, NPR, ND, P], f8, kind="ExternalInput")
    wp_d = nc.dram_tensor("wp8", [P, ND, D], f8, kind="ExternalInput")
    w1_d = nc.dram_tensor("w1", [P, NF, ND, P], bf16, kind="ExternalInput")
    w18_d = nc.dram_tensor("w18", [P, NF8, ND, P], f8, kind="ExternalInput")
    w2_d = nc.dram_tensor("w2", [P, NF, D], bf16, kind="ExternalInput")
    mask_d = nc.dram_tensor("mask", [P, P], f32, kind="ExternalInput")
    id_d = nc.dram_tensor("ident", [P, P], bf16, kind="ExternalInput")
    qb_d = kb_d = vb_d = b1_d = bp_d = b2_d = None
    if flags.get("has_qkb"):
        qb_d = nc.dram_tensor("qbt", [P, NPR], f32, kind="ExternalInput")
        kb_d = nc.dram_tensor("kbt", [P, NPR], f32, kind="ExternalInput")
    if flags.get("has_vb"):
        vb_d = nc.dram_tensor("vbr", [1, D], f32, kind="ExternalInput")
    if flags.get("has_b1"):
        b1_d = nc.dram_tensor("b1t", [P, NF], f32, kind="ExternalInput")
    if flags.get("has_bp"):
        bp_d = nc.dram_tensor("bpr", [1, D], f32, kind="ExternalInput")
    if flags.get("has_b2"):
        b2_d = nc.dram_tensor("b2r", [1, D], f32, kind="ExternalInput")
    x2_d = nc.dram_tensor("x2s", [T, D], f32, kind="Internal")
    out_d = nc.dram_tensor("out", [T, D], f32, kind="ExternalOutput")
    dbg = {}
    if debug:
        dbg["d_xnT"] = nc.dram_tensor("d_xnT", [P, ND, T], f8,
                                      kind="ExternalOutput")
        dbg["d_attT"] = nc.dram_tensor("d_attT", [P, ND, T], f8,
                                       kind="ExternalOutput")
        dbg["d_xn2T"] = nc.dram_tensor("d_xn2T", [P, ND, T], bf16,
                                       kind="ExternalOutput")
        dbg["d_q0"] = nc.dram_tensor("d_q0", [P, T], bf16,
                                     kind="ExternalOutput")
        dbg["d_v0"] = nc.dram_tensor("d_v0", [P, NT, P], bf16,
                                     kind="ExternalOutput")

    def bcast(ap_1d):
        return bass.AP(
            tensor=ap_1d.tensor,
            offset=ap_1d.offset,
            ap=[[0, P]] + list(ap_1d.ap)[1:],
        )

    with tile.TileContext(nc, pool_alloc_mode="queue") as tc, ExitStack() as top:
        const = top.enter_context(tc.tile_pool(name="const", bufs=1))
        mask_sb = const.tile([P, P], f32)
        id_sb = const.tile([P, P], bf16)
        eps_sb = const.tile([P, 1], f32)
        nc.vector.memset(eps_sb, EPS)
        qb_sb = kb_sb = vb_sb = b1_sb = bp_sb = b2_sb = None
        if qb_d is not None:
            qb_sb = const.tile([P, NPR], f32)
            kb_sb = const.tile([P, NPR], f32)
        if vb_d is not None:
            vb_sb = const.tile([P, D], f32)
        if b1_d is not None:
            b1_sb = const.tile([P, NF], f32)
        if bp_d is not None:
            bp_sb = const.tile([P, D], f32)
        if b2_d is not None:
            b2_sb = const.tile([P, D], f32)

        def const_dmas():
            nc.sync.dma_start(out=id_sb, in_=id_d[:, :])
            nc.sync.dma_start(out=mask_sb, in_=mask_d[:, :])
            if qb_sb is not None:
                nc.sync.dma_start(out=qb_sb, in_=qb_d[:, :])
                nc.sync.dma_start(out=kb_sb, in_=kb_d[:, :])
            if vb_sb is not None:
                nc.sync.dma_start(out=vb_sb, in_=bcast(vb_d[:, :]))
            if b1_sb is not None:
                nc.sync.dma_start(out=b1_sb, in_=b1_d[:, :])
            if bp_sb is not None:
                nc.sync.dma_start(out=bp_sb, in_=bcast(bp_d[:, :]))
            if b2_sb is not None:
                nc.sync.dma_start(out=b2_sb, in_=bcast(b2_d[:, :]))

        consts = dict(mask=mask_sb, ident=id_sb, eps=eps_sb, qb=qb_sb,
                      kb=kb_sb, vb=vb_sb, b1=b1_sb, bp=bp_sb, b2=b2_sb)
        drams = dict(x=x_d, wq=wq_d, wk=wk_d, wv=wv_d, wp=wp_d, w1=w1_d,
                     w18=w18_d, w2=w2_d, x2=x2_d, out=out_d)
        emit_args = (nc, tc, tile, bass, mybir, f32, bf16, f8, AF, ALU, DR,
                     drams, consts, scales, dbg, const_dmas, reps == 1)
        if reps == 1:
            _emit(*emit_args)
        else:
            u = unroll if reps >= unroll > 0 else 1
            n, rem = divmod(reps, u)
            with tc.For_i(0, n, 1):
                for _ in range(u):
                    _emit(*emit_args)
            for _ in range(rem):
                _emit(*emit_args)

    _split_multiwait_insts(nc, mybir)
    return nc


def _emit(nc, tc, tile, bass, mybir, f32, bf16, f8, AF, ALU, DR,
          drams, consts, scales, dbg, const_dmas, use_swdge=True):
    from contextlib import ExitStack

    x_d, wq_d, wk_d, wv_d = drams["x"], drams["wq"], drams["wk"], drams["wv"]
    wp_d, w1_d, w2_d = drams["wp"], drams["w1"], drams["w2"]
    w18_d = drams["w18"]
    x2_d, out_d = drams["x2"], drams["out"]
    mask_sb, id_sb, eps_sb = consts["mask"], consts["ident"], consts["eps"]
    qb_sb, kb_sb, vb_sb = consts["qb"], consts["kb"], consts["vb"]
    b1_sb, bp_sb, b2_sb = consts["b1"], consts["bp"], consts["b2"]
    inv_q, inv_k = scales["inv_q"], scales["inv_k"]
    inv_v, inv_proj = scales["inv_v"], scales["inv_proj"]
    inv_1 = scales["inv_1"]

    big_dma = nc.gpsimd if use_swdge else nc.sync

    with ExitStack() as ctx:
        big = ctx.enter_context(tc.tile_pool(name="big", bufs=1))
        xnT = big.tile([P, ND, T], f8)       # LN1(x)^T, fp8   [d, t]
        attT = big.tile([P, ND, T], f8)      # attn out^T *16  [e', t]
        xn2T = big.tile([P, ND, T], bf16)    # LN2(x2)^T       [d, t]
        wp_sb = big.tile([P, ND, D], f8)
        w2_sb = big.tile([P, NF, D], bf16)
        h0 = big.tile([P, NF, 512], bf16)    # FF1 half output [f, t-half]

        wqkp = ctx.enter_context(tc.tile_pool(name="wqkp", bufs=2))
        qkp = ctx.enter_context(tc.tile_pool(name="qkp", bufs=2))
        vrp = ctx.enter_context(tc.tile_pool(name="vrp", bufs=2))
        wexpp = ctx.enter_context(tc.tile_pool(name="wexpp", bufs=3))
        smal = ctx.enter_context(tc.tile_pool(name="smal", bufs=2))
        xsp = ctx.enter_context(tc.tile_pool(name="xsp", bufs=3))
        xcp = ctx.enter_context(tc.tile_pool(name="xcp", bufs=2))
        x2p = ctx.enter_context(tc.tile_pool(name="x2p", bufs=2))
        w1p = ctx.enter_context(tc.tile_pool(name="w1p", bufs=3))
        otp = ctx.enter_context(tc.tile_pool(name="otp", bufs=2))
        stats = ctx.enter_context(tc.tile_pool(name="stats", bufs=4))

        psS = ctx.enter_context(tc.tile_pool(name="psS", bufs=2, space="PSUM"))
        psM = ctx.enter_context(tc.tile_pool(name="psM", bufs=2, space="PSUM"))
        psA = ctx.enter_context(tc.tile_pool(name="psA", bufs=1, space="PSUM"))

        def ln_stats(xin, tg):
            st = stats.tile([P, 2, 6], f32, tag=tg + "st")
            nc.vector.bn_stats(out=st[:, 0, :], in_=xin[:, 0:512])
            nc.vector.bn_stats(out=st[:, 1, :], in_=xin[:, 512:1024])
            mv = stats.tile([P, 2], f32, tag=tg + "mv")
            nc.vector.bn_aggr(out=mv, in_=st)
            sd = stats.tile([P, 1], f32, tag=tg + "sd")
            nc.scalar.activation(sd, mv[:, 1:2], AF.Sqrt, bias=eps_sb)
            rs = stats.tile([P, 1], f32, tag=tg + "rs")
            nc.vector.reciprocal(out=rs, in_=sd)
            nmu = stats.tile([P, 1], f32, tag=tg + "nmu")
            nc.vector.tensor_scalar(
                out=nmu, in0=mv[:, 0:1], scalar1=rs, scalar2=-1.0,
                op0=ALU.mult, op1=ALU.mult)
            return rs, nmu

        def ln_tile(xin, xcout, tg):
            rs, nmu = ln_stats(xin, tg)
            nc.scalar.activation(xcout, xin, AF.Identity, bias=nmu, scale=rs)

        def transpose_tile(xc, dst, i):
            # 8 [P,P] PE transposes as 2 quads, one psum copy per quad
            for q in range(2):
                tag = "scA" if q == 0 else "scB"
                tp = psS.tile([P, 4, P], bf16, tag=tag)
                for k in range(4):
                    j = 4 * q + k
                    nc.tensor.transpose(
                        tp[:, k, :], xc[:, P * j:P * (j + 1)], id_sb)
                cp = nc.vector.tensor_copy if q == 0 else nc.scalar.copy
                cp(out=dst[:, 4 * q:4 * (q + 1), P * i:P * (i + 1)], in_=tp)

        def qkv_make(pr):
            """DMA pair-pr weights; return (tiles, PE filler closures)."""
            wqt = wqkp.tile([P, ND, P], f8, tag="wq")
            wkt = wqkp.tile([P, ND, P], f8, tag="wk")
            wvt = wqkp.tile([P, ND, P], f8, tag="wv")
            nc.sync.dma_start(out=wqt, in_=wq_d[:, pr, :, :])
            nc.sync.dma_start(out=wkt, in_=wk_d[:, pr, :, :])
            nc.sync.dma_start(out=wvt, in_=wv_d[:, pr, :, :])
            qTt = qkp.tile([P, T], bf16, tag="qT")
            kTt = qkp.tile([P, T], bf16, tag="kT")
            vt = vrp.tile([P, NT, P], bf16, tag="v")
            fillers = []
            for wt, dstt, invs, bias in (
                (wqt, qTt, inv_q, qb_sb), (wkt, kTt, inv_k, kb_sb)
            ):
                for c in range(4):
                    def fqk(wt=wt, dstt=dstt, invs=invs, bias=bias, c=c,
                            pr=pr):
                        ps = psM.tile([P, 512], f32, tag="mm")
                        for j in range(4):
                            nc.tensor.matmul(
                                ps[:, 0:256], wt[:, 2 * j:2 * j + 2, :],
                                xnT[:, 2 * j:2 * j + 2, 256 * c:256 * (c + 1)],
                                start=(j == 0), stop=(j == 3), perf_mode=DR)
                        if bias is not None:
                            nc.vector.tensor_scalar(
                                out=dstt[:, 256 * c:256 * (c + 1)],
                                in0=ps[:, 0:256], scalar1=invs,
                                scalar2=bias[:, pr:pr + 1],
                                op0=ALU.mult, op1=ALU.add)
                        else:
                            nc.vector.tensor_scalar_mul(
                                out=dstt[:, 256 * c:256 * (c + 1)],
                                in0=ps[:, 0:256], scalar1=invs)
                    fillers.append(fqk)
            for i in range(NT):
                def fv(i=i, wvt=wvt, vt=vt, pr=pr):
                    ps = psM.tile([P, 512], f32, tag="mm")
                    for j in range(4):
                        nc.tensor.matmul(
                            ps[:, 0:128], xnT[:, 2 * j:2 * j + 2,
                                              P * i:P * (i + 1)],
                            wvt[:, 2 * j:2 * j + 2, :],
                            start=(j == 0), stop=(j == 3), perf_mode=DR)
                    nc.vector.tensor_scalar_mul(
                        out=vt[:, i, :], in0=ps[:, 0:128], scalar1=inv_v)
                    if vb_sb is not None:
                        nc.vector.tensor_add(
                            out=vt[:, i, :], in0=vt[:, i, :],
                            in1=vb_sb[:, P * pr:P * (pr + 1)])
                fillers.append(fv)
            return (qTt, kTt, vt), fillers

        def emit_pair(pr, cur, nxt_fillers):
            qTt, kTt, vt = cur
            attps = psA.tile([P, T], f32, tag="att")
            sums = smal.tile([P, 2, NT, 2], f32, tag="sums")
            rr = smal.tile([P, 2, NT], f32, tag="rr")
            wexps = [None] * NT
            fill = list(nxt_fillers)
            fi = 0

            def scores(i):
                # 512-col psum chunks, double-buffered per head: exp(i)
                # overlaps the next score matmuls with no psum stall
                r0 = P * i
                rlen = T - r0
                wx = wexpp.tile([P, 2, T], bf16, tag="wexp")
                wexps[i] = wx
                for hb in range(2):
                    for ci, c0 in enumerate(range(0, rlen, 512)):
                        cl = min(512, rlen - c0)
                        sc = psS.tile([P, 512], f32,
                                      tag=("scA" if hb == 0 else "scB"))
                        nc.tensor.matmul(
                            sc[:, 0:cl],
                            kTt[64 * hb:64 * (hb + 1), r0:r0 + P],
                            qTt[64 * hb:64 * (hb + 1),
                                r0 + c0:r0 + c0 + cl],
                            start=True, stop=True,
                            tile_position=(64 * hb, 0))
                        if ci == 0:
                            nc.vector.tensor_add(
                                out=sc[:, 0:P], in0=sc[:, 0:P], in1=mask_sb)
                        nc.scalar.activation(
                            wx[:, hb, c0:c0 + cl], sc[:, 0:cl], AF.Exp,
                            scale=SCALE,
                            accum_out=sums[:, hb, i, ci:ci + 1])
                if rlen > 512:
                    nc.vector.tensor_add(
                        out=sums[:, :, i, 0:1], in0=sums[:, :, i, 0:1],
                        in1=sums[:, :, i, 1:2])

            def av(i):
                r0 = P * i
                nc.vector.reciprocal(
                    out=rr[:, :, i:i + 1], in_=sums[:, :, i, 0:1])
                vpt = smal.tile([P, 2, DH], bf16, tag="vp")
                for hb in range(2):
                    nc.vector.tensor_scalar_mul(
                        out=vpt[:, hb, :],
                        in0=vt[:, i, DH * hb:DH * (hb + 1)],
                        scalar1=rr[:, hb, i:i + 1])
                wx = wexps[i]
                for lo, hi in _av_chunks(r0):
                    bank = lo // 512
                    last_i = min(NT - 1, 4 * bank + 3)
                    for hb in range(2):
                        nc.tensor.matmul(
                            attps[64 * hb:64 * (hb + 1), lo:hi],
                            vpt[:, hb, :],
                            wx[:, hb, lo - r0:hi - r0],
                            start=(i == 0), stop=(i == last_i),
                            tile_position=(0, 64 * hb))

            def att_copy(half):
                c0 = 512 * half
                nc.vector.tensor_scalar_mul(
                    out=attT[:, pr, c0:c0 + 512],
                    in0=attps[:, c0:c0 + 512], scalar1=SATT)

            scores(0)
            scores(1)
            for i in range(2, NT):
                for _ in range(2):
                    if fi < len(fill):
                        fill[fi]()
                        fi += 1
                scores(i)
                av(i - 2)
                if i == 5:
                    att_copy(0)
            av(6)
            av(7)
            while fi < len(fill):
                fill[fi]()
                fi += 1
            att_copy(1)

        # ================= Phase A =================
        xts = []
        for i in range(NT):
            xt = xsp.tile([P, D], f32, tag="xs")
            nc.sync.dma_start(out=xt[:, 0:512], in_=x_d[P * i:P * (i + 1), 0:512])
            nc.sync.dma_start(out=xt[:, 512:1024],
                              in_=x_d[P * i:P * (i + 1), 512:1024])
            xts.append(xt)
            if i == 0:
                const_dmas()
        cur, fillers = qkv_make(0)  # pair-0 weight DMAs go out early
        # bulk weights on swdge, early (consumed in C/E)
        for k in range(ND):
            big_dma.dma_start(out=wp_sb[:, k, :], in_=wp_d[:, k, :])
        for k in range(NF):
            big_dma.dma_start(out=w2_sb[:, k, :], in_=w2_d[:, k, :])

        # LN stats for all tiles first (breaks the per-tile serial chain),
        # then applies + transposes with pair-0 qkv matmuls interleaved:
        # qk chunk c needs xnT t-tiles {2c, 2c+1} only, v(i) needs tile i.
        lns = [ln_stats(xts[i], f"a{i % 2}") for i in range(NT)]
        for i in range(NT):
            xc = xcp.tile([P, D], bf16, tag="xc")
            rs, nmu = lns[i]
            nc.scalar.activation(xc, xts[i], AF.Identity, bias=nmu, scale=rs)
            transpose_tile(xc, xnT, i)
            if i % 2 == 1:
                c = (i - 1) // 2
                fillers[c]()          # q chunk c
                fillers[4 + c]()      # k chunk c
                fillers[8 + i - 1]()  # v(i-1)
                fillers[8 + i]()      # v(i)
        xts = None

        # ================= Phase B =================
        if dbg:
            nc.gpsimd.dma_start(out=dbg["d_q0"][:, :], in_=cur[0])
            nc.gpsimd.dma_start(out=dbg["d_v0"][:, :, :], in_=cur[2])
        for pr in range(NPR):
            if pr + 1 < NPR:
                nxt, nfill = qkv_make(pr + 1)
            else:
                nxt, nfill = None, []
            emit_pair(pr, cur, nfill)
            cur = nxt

        # ================= Phase C =================
        for m in range(NT):
            xr = xsp.tile([P, D], f32, tag="xs")
            nc.sync.dma_start(out=xr, in_=x_d[P * m:P * (m + 1), :])
            x2t = x2p.tile([P, D], f32, tag="x2t")
            for c in range(4):
                ps = psM.tile([P, 512], f32, tag="mm")
                for j in range(4):
                    nc.tensor.matmul(
                        ps[:, 0:256],
                        attT[:, 2 * j:2 * j + 2, P * m:P * (m + 1)],
                        wp_sb[:, 2 * j:2 * j + 2, 256 * c:256 * (c + 1)],
                        start=(j == 0), stop=(j == 3), perf_mode=DR)
                nc.vector.scalar_tensor_tensor(
                    out=x2t[:, 256 * c:256 * (c + 1)], in0=ps[:, 0:256],
                    scalar=inv_proj, op0=ALU.mult,
                    in1=xr[:, 256 * c:256 * (c + 1)], op1=ALU.add)
            if bp_sb is not None:
                nc.vector.tensor_add(out=x2t, in0=x2t, in1=bp_sb)
            nc.sync.dma_start(out=x2_d[P * m:P * (m + 1), :], in_=x2t)
            xc2 = xcp.tile([P, D], bf16, tag="xc")
            ln_tile(x2t, xc2, "c")
            transpose_tile(xc2, xn2T, m)

        if dbg:
            nc.gpsimd.dma_start(out=dbg["d_xnT"][:, :, :], in_=xnT)
            nc.gpsimd.dma_start(out=dbg["d_attT"][:, :, :], in_=attT)
            nc.gpsimd.dma_start(out=dbg["d_xn2T"][:, :, :], in_=xn2T)

        # ============ Phases D/E interleaved by t-half ============
        for n in range(2):
            for mf in range(NF):
                w1t = w1p.tile([P, ND, P], bf16, tag="w1t")
                big_dma.dma_start(out=w1t, in_=w1_d[:, mf, :, :])
                ps = psM.tile([P, 512], f32, tag="mm")
                for k in range(ND):
                    nc.tensor.matmul(
                        ps, w1t[:, k, :], xn2T[:, k, 512 * n:512 * (n + 1)],
                        start=(k == 0), stop=(k == ND - 1))
                dst = h0[:, mf, :]
                use_act = (mf % 2 == 0)
                if b1_sb is not None:
                    if use_act:
                        nc.scalar.activation(dst, ps, AF.Relu,
                                             bias=b1_sb[:, mf:mf + 1])
                    else:
                        nc.vector.tensor_scalar(
                            out=dst, in0=ps, scalar1=b1_sb[:, mf:mf + 1],
                            scalar2=0.0, op0=ALU.add, op1=ALU.max)
                else:
                    if use_act:
                        nc.scalar.activation(dst, ps, AF.Relu)
                    else:
                        nc.vector.tensor_scalar_max(out=dst, in0=ps,
                                                    scalar1=0.0)
            for mloc in range(4):
                m = 4 * n + mloc
                x2r = xsp.tile([P, D], f32, tag="xs")
                nc.sync.dma_start(out=x2r, in_=x2_d[P * m:P * (m + 1), :])
                for c in range(2):
                    ps = psM.tile([P, 512], f32, tag="mm")
                    for kf in range(NF):
                        nc.tensor.matmul(
                            ps, h0[:, kf, P * mloc:P * (mloc + 1)],
                            w2_sb[:, kf, 512 * c:512 * (c + 1)],
                            start=(kf == 0), stop=(kf == NF - 1))
                    ot = otp.tile([P, 512], f32, tag="ot")
                    nc.vector.tensor_add(
                        out=ot, in0=x2r[:, 512 * c:512 * (c + 1)], in1=ps)
                    if b2_sb is not None:
                        nc.vector.tensor_add(
                            out=ot, in0=ot,
                            in1=b2_sb[:, 512 * c:512 * (c + 1)])
                    nc.sync.dma_start(
                        out=out_d[P * m:P * (m + 1),
                                  512 * c:512 * (c + 1)], in_=ot)


def get_nc(reps=1, flags=None, scales=None, debug=False, unroll=1):
    flags = flags or {}
    scales = scales or {"inv_q": 1.0, "inv_k": 1.0, "inv_v": 1.0,
                        "inv_proj": 1.0 / SATT, "inv_1": 1.0}
    key = (reps, tuple(sorted(flags.items())),
           tuple(sorted(scales.items())), debug, unroll)
    if key not in _cache:
        _cache[key] = _build(reps=reps, flags=flags, scales=scales,
                             debug=debug, unroll=unroll)
    return _cache[key]


def run(x, common, flags, scales, reps=1, debug=False, unroll=1):
    from concourse.bass_utils import run_bass_kernel_spmd

    nc = get_nc(reps=reps, flags=flags, scales=scales, debug=debug,
                unroll=unroll)
    in_maps = [dict(common, x=np.ascontiguousarray(x[c])) for c in range(B)]
    return run_bass_kernel_spmd(nc, in_maps, core_ids=list(range(B)))


def kernel(x, gamma1, beta1, Wq, Wk, Wv, Wp, bp, gamma2, beta2, W1, b1, W2, b2):
    xs, common, flags, scales = _prep_inputs(
        x, gamma1, beta1, Wq, Wk, Wv, Wp, bp, gamma2, beta2, W1, b1, W2, b2
    )
    res = run(xs, common, flags, scales, reps=1)
    out = np.stack([res.results[c]["out"] for c in range(B)], axis=0)
    return out.astype(np.float32)
